# revision 17
# baseline (speedup 1.0000x reference)
"""Euclidean distance block (retrieval kNN) on 8 TRN2 NeuronCores.

dist[b, s, p] = sqrt(sum_c (x1[b, c, p] - x2[b, s, c, p])^2)   p = spatial (h*w)
out[b] = dist[b].reshape(S * h * w)

Sharding: data-parallel over batch B=32 -> 4 batches per core, no comms.

Per-core layout (spatial-split): SBUF partitions carry (channel, spatial_half)
= 64*2 = 128; the free axis carries the 882 spatial positions of one half.
Every tensor reshapes cleanly onto 128 partitions:
  x1[b]  -> [128, 882]      one DMA per batch, no partition-duplicate copy
  x2[b,s]-> [128, 882]      streamed as [128, 2, 882] two-support tiles
  out[b] -> rows (s, si)    [50, 882] f32, contiguous 3528 B per row

Compute chain per support: DVE subtract in bf16 (2x mode, in place), Square
split between ACT and DVE (cadence balance), then one [128, 50] one-hot mask
matmul per spatial half accumulating sum-over-C into PSUM [50, 441] (rows =
(s, si)); supports accumulate start/stop over s = 0..24.  LDWEIGHTS streams on
PE's second read port so per-support mask reloads pipeline behind the matmuls.
ACT Sqrt PSUM -> SBUF f32, one 176 KB store per batch on the otherwise-idle
Sync HWDGE ring (loads-with-cast must use the GpSimd SWDGE ring, and a store
queued behind loads on the same ring would stall them).
"""

import numpy as np

B, S, C, H, W = 32, 25, 64, 42, 42
HW = H * W            # 1764
PHALF = HW // 2       # 882
QW = PHALF // 2       # 441
NCORES = 8
BL = B // NCORES      # 4 batches per core
NSO = 2               # supports per streamed tile
NTILE = S // NSO      # 12 two-support tiles, then 1 leftover single

_cache = {}


def _build_nc():
    import concourse.bacc as bacc
    import concourse.mybir as mybir
    from concourse.tile import TileContext
    from concourse.bass import MemorySpace

    f32 = mybir.dt.float32
    bf16 = mybir.dt.bfloat16
    Square = mybir.ActivationFunctionType.Square
    Sqrt = mybir.ActivationFunctionType.Sqrt
    sub = mybir.AluOpType.subtract
    mult = mybir.AluOpType.mult

    # Square and Sqrt both live in the "sqrt_and_others" act-function set,
    # but the table-load chooser picks the first set containing each one,
    # alternating two ~2.7us table reloads per batch. Strip the two
    # functions from every other set (contents only — set ids are
    # positional) so one resident table serves the whole kernel.
    _orig_tables = bacc.get_activation_tables

    def _pinned_tables(arch):
        t = _orig_tables(arch)
        for name, fns in t.items():
            if name != "sqrt_and_others":
                fns.discard(Square)
                fns.discard(Sqrt)
        return t

    bacc.get_activation_tables = _pinned_tables
    nc = bacc.Bacc()
    # x1/x2 are staged to DRAM as bf16 by make_in_maps: the kernel is
    # HBM-stream-bound, so halving the bytes read halves the floor; the
    # compute pipeline already ran on bf16 (identical numerics to the
    # previous cast-during-DMA scheme).
    x1 = nc.declare_dram_parameter("x1", [BL, C, HW], bf16, isOutput=False)
    x2 = nc.declare_dram_parameter("x2", [BL, S, C, HW], bf16, isOutput=False)
    mk = nc.declare_dram_parameter("mask", [128, S, 2 * S], bf16, isOutput=False)
    out = nc.declare_dram_parameter("out", [BL, S * HW], f32, isOutput=True)

    with TileContext(nc) as tc:
        with (
            tc.tile_pool(name="x2p", bufs=16) as x2p,
            tc.tile_pool(name="sqp", bufs=8) as sqp,
            tc.tile_pool(name="x1p", bufs=2) as x1p,
            tc.tile_pool(name="outp", bufs=2) as outp,
            tc.tile_pool(name="cst", bufs=1) as cst,
            tc.tile_pool(name="ps", bufs=3, space=MemorySpace.PSUM) as psp,
        ):
            # mask rows are (s, si): mt[k, s, 2s+si(k)] = 1; contiguous
            # 2500 B per partition, one clean HWDGE load
            mt = cst.tile([128, S, 2 * S], bf16)
            nc.sync.dma_start(mt[:], mk[:, :, :])

            prev = None
            for b in range(BL):
                # x1[b]: partition (c, si), one contiguous 3528 B run per
                # partition, cast-loaded just ahead of its batch's stream
                x1bt = x1p.tile([128, PHALF], bf16, tag="x1b")
                # b0's x1 + first two tiles ride the Sync HWDGE ring: its
                # RTL descgen delivers them ~4us before the gpsimd Q7 has
                # warmed up, and the pipeline is compute-bound, so an
                # earlier start is an earlier finish
                x1ring = nc.sync if b == 0 else nc.gpsimd
                x1ring.dma_start(
                    x1bt[:], x1[b].rearrange("c (si p) -> (c si) p", si=2)
                )
                # stream all supports of the batch: 12 two-support tiles
                # plus the odd support 24 as a single-support tile
                tiles = []
                for i in range(NTILE):
                    x2t = x2p.tile([128, NSO, PHALF], bf16, tag="x2t")
                    ring = nc.sync if (b == 0 and i < 2) else nc.gpsimd
                    ring.dma_start(
                        x2t[:],
                        x2[b, NSO * i : NSO * (i + 1)].rearrange(
                            "s c (si p) -> (c si) s p", si=2
                        ),
                    )
                    tiles.append(x2t)
                # the odd support shares the x2t pool tag so the scheduler
                # keeps its (small, always-slot-ready) DMA in stream order
                # instead of hoisting it to the front of the ring
                x2l = x2p.tile([128, NSO, PHALF], bf16, tag="x2t")
                nc.gpsimd.dma_start(
                    x2l[:, 0:1, :],
                    x2[b, S - 1 :].rearrange("s c (si p) -> (c si) s p", si=2),
                )

                x1b = x1bt[:, :]
                pst = [
                    psp.tile([2 * S, QW], f32, name=f"ps{h}", tag=f"ps{h}")
                    for h in range(2)
                ]

                def flush_prev(prev=prev):
                    # emit the previous batch's sqrt + store mid-way into
                    # this batch's loop: by then its last matmul has long
                    # retired, so ACT slots the sqrts between squares
                    # instead of head-of-line blocking on the accumulation
                    if prev is None:
                        return
                    ppst, pb = prev
                    ot = outp.tile([2 * S, PHALF], f32, name="ot", tag="ot")
                    for h in range(2):
                        nc.scalar.activation(
                            ot[:, h * QW : (h + 1) * QW], ppst[h][:], Sqrt
                        )
                    nc.sync.dma_start(
                        out[pb].rearrange("(s si p) -> (s si) p", si=2, p=PHALF),
                        ot[:],
                    )

                x1bc = x1b.unsqueeze(1).broadcast_to([128, NSO, PHALF])
                for s in range(S):
                    if s == 4:
                        flush_prev()
                    if s < S - 1:
                        d = tiles[s // NSO][:, s % NSO, :]
                        if s % NSO == 0:
                            # one fused subtract per tile: x1 broadcast
                            # (stride-0) over the support dim halves the
                            # DVE op count
                            t = tiles[s // NSO]
                            nc.vector.tensor_tensor(t[:], t[:], x1bc, sub)
                    else:
                        d = x2l[:, 0, :]
                        nc.vector.tensor_tensor(d, d, x1b, sub)
                    sq = sqp.tile([128, PHALF], bf16, tag="sq")
                    # squares split 2/1 ACT/DVE to balance engine cadence
                    if s % 3 != 2:
                        nc.scalar.activation(sq[:], d, Square)
                    else:
                        nc.vector.tensor_tensor(sq[:], d, d, mult)
                    for h in range(2):
                        nc.tensor.matmul(
                            pst[h][:, :],
                            mt[:, s, :],
                            sq[:, h * QW : (h + 1) * QW],
                            start=(s == 0),
                            stop=(s == S - 1),
                        )
                prev = (pst, b)

            # final batch: sqrt + store immediately, in quarter-chunks with
            # stores alternating between the Sync and Scalar rings so their
            # descriptor generation runs in parallel
            ppst, pb = prev
            ot = outp.tile([2 * S, PHALF], f32, name="ot", tag="ot")
            dst = out[pb].rearrange("(s si p) -> (s si) p", si=2, p=PHALF)
            HQ = QW // 2
            for c in range(4):
                h, q = c // 2, c % 2
                w = HQ if q == 0 else QW - HQ
                osl = slice(h * QW + q * HQ, h * QW + q * HQ + w)
                psl = slice(q * HQ, q * HQ + w)
                nc.scalar.activation(ot[:, osl], ppst[h][:, psl], Sqrt)
                (nc.sync if c % 2 == 0 else nc.scalar).dma_start(
                    dst[:, osl], ot[:, osl]
                )

    try:
        nc.finalize()
    finally:
        bacc.get_activation_tables = _orig_tables
    return nc


def get_nc():
    if "nc" not in _cache:
        _cache["nc"] = _build_nc()
    return _cache["nc"]


def make_mask() -> np.ndarray:
    # mask[k, s, m] = 1 iff the sum of partition k (= channel c = k//2,
    # spatial half si = k%2) for support s belongs to output row m = 2s + si.
    import ml_dtypes

    mask = np.zeros((128, S, 2 * S), dtype=ml_dtypes.bfloat16)
    for k in range(128):
        si = k % 2
        for s in range(S):
            mask[k, s, 2 * s + si] = 1.0
    return mask


def make_in_maps(x1: np.ndarray, x2: np.ndarray) -> list[dict]:
    import ml_dtypes

    bf = ml_dtypes.bfloat16
    x1 = np.ascontiguousarray(np.asarray(x1).astype(bf)).reshape(B, C, HW)
    x2 = np.ascontiguousarray(np.asarray(x2).astype(bf)).reshape(B, S, C, HW)
    mask = make_mask()
    maps = []
    for i in range(NCORES):
        sl = slice(i * BL, (i + 1) * BL)
        maps.append({"x1": x1[sl], "x2": x2[sl], "mask": mask})
    return maps


def gather_out(results: list[dict]) -> np.ndarray:
    return np.concatenate([np.asarray(r["out"]) for r in results], axis=0).astype(
        np.float32, copy=False
    )


def kernel(x1, x2) -> np.ndarray:
    from concourse.bass_utils import run_bass_kernel_spmd

    nc = get_nc()
    in_maps = make_in_maps(x1, x2)
    res = run_bass_kernel_spmd(nc, in_maps, list(range(NCORES)))
    return gather_out(res.results)


# revision 18
# speedup vs baseline: 1.0590x; 1.0590x over previous
"""Euclidean distance block (retrieval kNN) on 8 TRN2 NeuronCores.

dist[b, s, p] = sqrt(sum_c (x1[b, c, p] - x2[b, s, c, p])^2)   p = spatial (h*w)
out[b] = dist[b].reshape(S * h * w)

Sharding: data-parallel over batch B=32 -> 4 batches per core, no comms.

Per-core layout (spatial-split): SBUF partitions carry (channel, spatial_half)
= 64*2 = 128; the free axis carries the 882 spatial positions of one half.
Every tensor reshapes cleanly onto 128 partitions:
  x1[b]  -> [128, 882]      one DMA per batch, no partition-duplicate copy
  x2[b,s]-> [128, 882]      streamed as [128, 2, 882] two-support tiles
  out[b] -> rows (s, si)    [50, 882] f32, contiguous 3528 B per row

Compute chain per support: DVE subtract in bf16 (2x mode, in place), Square
split between ACT and DVE (cadence balance), then one [128, 50] one-hot mask
matmul per spatial half accumulating sum-over-C into PSUM [50, 441] (rows =
(s, si)); supports accumulate start/stop over s = 0..24.  LDWEIGHTS streams on
PE's second read port so per-support mask reloads pipeline behind the matmuls.
ACT Sqrt PSUM -> SBUF f32, one 176 KB store per batch on the otherwise-idle
Sync HWDGE ring (loads-with-cast must use the GpSimd SWDGE ring, and a store
queued behind loads on the same ring would stall them).
"""

import numpy as np

B, S, C, H, W = 32, 25, 64, 42, 42
HW = H * W            # 1764
PHALF = HW // 2       # 882
QW = PHALF // 2       # 441
NCORES = 8
BL = B // NCORES      # 4 batches per core
NSO = 2               # supports per streamed tile
NTILE = S // NSO      # 12 two-support tiles, then 1 leftover single

_cache = {}


def _build_nc():
    import concourse.bacc as bacc
    import concourse.mybir as mybir
    from concourse.tile import TileContext
    from concourse.bass import MemorySpace

    f32 = mybir.dt.float32
    bf16 = mybir.dt.bfloat16
    Square = mybir.ActivationFunctionType.Square
    Sqrt = mybir.ActivationFunctionType.Sqrt
    sub = mybir.AluOpType.subtract
    mult = mybir.AluOpType.mult

    # Square and Sqrt both live in the "sqrt_and_others" act-function set,
    # but the table-load chooser picks the first set containing each one,
    # alternating two ~2.7us table reloads per batch. Strip the two
    # functions from every other set (contents only — set ids are
    # positional) so one resident table serves the whole kernel.
    _orig_tables = bacc.get_activation_tables

    def _pinned_tables(arch):
        t = _orig_tables(arch)
        for name, fns in t.items():
            if name != "sqrt_and_others":
                fns.discard(Square)
                fns.discard(Sqrt)
        return t

    bacc.get_activation_tables = _pinned_tables
    nc = bacc.Bacc()
    # x1/x2 are staged to DRAM as bf16 by make_in_maps: the kernel is
    # HBM-stream-bound, so halving the bytes read halves the floor; the
    # compute pipeline already ran on bf16 (identical numerics to the
    # previous cast-during-DMA scheme).
    x1 = nc.declare_dram_parameter("x1", [BL, C, HW], bf16, isOutput=False)
    x2 = nc.declare_dram_parameter("x2", [BL, S, C, HW], bf16, isOutput=False)
    mk = nc.declare_dram_parameter("mask", [128, S, 2 * S], bf16, isOutput=False)
    out = nc.declare_dram_parameter("out", [BL, S * HW], f32, isOutput=True)

    with TileContext(nc) as tc:
        with (
            tc.tile_pool(name="x2p", bufs=16) as x2p,
            tc.tile_pool(name="sqp", bufs=8) as sqp,
            tc.tile_pool(name="x1p", bufs=2) as x1p,
            tc.tile_pool(name="outp", bufs=2) as outp,
            tc.tile_pool(name="cst", bufs=1) as cst,
            tc.tile_pool(name="ps", bufs=3, space=MemorySpace.PSUM) as psp,
        ):
            # mask rows are (s, si): mt[k, s, 2s+si(k)] = 1; contiguous
            # 2500 B per partition, one clean HWDGE load
            mt = cst.tile([128, S, 2 * S], bf16)
            nc.sync.dma_start(mt[:], mk[:, :, :])

            prev = None
            for b in range(BL):
                # x1[b]: partition (c, si), one contiguous 3528 B run per
                # partition, cast-loaded just ahead of its batch's stream
                x1bt = x1p.tile([128, PHALF], bf16, tag="x1b")
                nc.gpsimd.dma_start(
                    x1bt[:], x1[b].rearrange("c (si p) -> (c si) p", si=2)
                )
                # stream all supports of the batch: 12 two-support tiles
                # plus the odd support 24 as a single-support tile
                tiles = []
                for i in range(NTILE):
                    x2t = x2p.tile([128, NSO, PHALF], bf16, tag="x2t")
                    nc.gpsimd.dma_start(
                        x2t[:],
                        x2[b, NSO * i : NSO * (i + 1)].rearrange(
                            "s c (si p) -> (c si) s p", si=2
                        ),
                    )
                    tiles.append(x2t)
                # the odd support shares the x2t pool tag so the scheduler
                # keeps its (small, always-slot-ready) DMA in stream order
                # instead of hoisting it to the front of the ring
                x2l = x2p.tile([128, NSO, PHALF], bf16, tag="x2t")
                nc.gpsimd.dma_start(
                    x2l[:, 0:1, :],
                    x2[b, S - 1 :].rearrange("s c (si p) -> (c si) s p", si=2),
                )

                x1b = x1bt[:, :]
                pst = [
                    psp.tile([2 * S, QW], f32, name=f"ps{h}", tag=f"ps{h}")
                    for h in range(2)
                ]

                def flush_prev(prev=prev):
                    # emit the previous batch's sqrt + store mid-way into
                    # this batch's loop: by then its last matmul has long
                    # retired, so ACT slots the sqrts between squares
                    # instead of head-of-line blocking on the accumulation
                    if prev is None:
                        return
                    ppst, pb = prev
                    ot = outp.tile([2 * S, PHALF], f32, name="ot", tag="ot")
                    for h in range(2):
                        nc.scalar.activation(
                            ot[:, h * QW : (h + 1) * QW], ppst[h][:], Sqrt
                        )
                    nc.sync.dma_start(
                        out[pb].rearrange("(s si p) -> (s si) p", si=2, p=PHALF),
                        ot[:],
                    )

                x1bc = x1b.unsqueeze(1).broadcast_to([128, NSO, PHALF])
                for s in range(S):
                    if s == 4:
                        flush_prev()
                    if s < S - 1:
                        d = tiles[s // NSO][:, s % NSO, :]
                        if s % NSO == 0:
                            # one fused subtract per tile: x1 broadcast
                            # (stride-0) over the support dim halves the
                            # DVE op count
                            t = tiles[s // NSO]
                            nc.vector.tensor_tensor(t[:], t[:], x1bc, sub)
                    else:
                        d = x2l[:, 0, :]
                        nc.vector.tensor_tensor(d, d, x1b, sub)
                    sq = sqp.tile([128, PHALF], bf16, tag="sq")
                    # squares split 2/1 ACT/DVE to balance engine cadence
                    if s % 3 != 2:
                        nc.scalar.activation(sq[:], d, Square)
                    else:
                        nc.vector.tensor_tensor(sq[:], d, d, mult)
                    for h in range(2):
                        nc.tensor.matmul(
                            pst[h][:, :],
                            mt[:, s, :],
                            sq[:, h * QW : (h + 1) * QW],
                            start=(s == 0),
                            stop=(s == S - 1),
                        )
                prev = (pst, b)

            # final batch: sqrt + store immediately, in quarter-chunks with
            # stores alternating between the Sync and Scalar rings so their
            # descriptor generation runs in parallel
            ppst, pb = prev
            ot = outp.tile([2 * S, PHALF], f32, name="ot", tag="ot")
            dst = out[pb].rearrange("(s si p) -> (s si) p", si=2, p=PHALF)
            HQ = QW // 2
            for c in range(4):
                h, q = c // 2, c % 2
                w = HQ if q == 0 else QW - HQ
                osl = slice(h * QW + q * HQ, h * QW + q * HQ + w)
                psl = slice(q * HQ, q * HQ + w)
                nc.scalar.activation(ot[:, osl], ppst[h][:, psl], Sqrt)
                (nc.sync if c % 2 == 0 else nc.scalar).dma_start(
                    dst[:, osl], ot[:, osl]
                )

    try:
        nc.finalize()
    finally:
        bacc.get_activation_tables = _orig_tables
    return nc


def get_nc():
    if "nc" not in _cache:
        _cache["nc"] = _build_nc()
    return _cache["nc"]


def make_mask() -> np.ndarray:
    # mask[k, s, m] = 1 iff the sum of partition k (= channel c = k//2,
    # spatial half si = k%2) for support s belongs to output row m = 2s + si.
    import ml_dtypes

    mask = np.zeros((128, S, 2 * S), dtype=ml_dtypes.bfloat16)
    for k in range(128):
        si = k % 2
        for s in range(S):
            mask[k, s, 2 * s + si] = 1.0
    return mask


def make_in_maps(x1: np.ndarray, x2: np.ndarray) -> list[dict]:
    import ml_dtypes

    bf = ml_dtypes.bfloat16
    x1 = np.ascontiguousarray(np.asarray(x1).astype(bf)).reshape(B, C, HW)
    x2 = np.ascontiguousarray(np.asarray(x2).astype(bf)).reshape(B, S, C, HW)
    mask = make_mask()
    maps = []
    for i in range(NCORES):
        sl = slice(i * BL, (i + 1) * BL)
        maps.append({"x1": x1[sl], "x2": x2[sl], "mask": mask})
    return maps


def gather_out(results: list[dict]) -> np.ndarray:
    return np.concatenate([np.asarray(r["out"]) for r in results], axis=0).astype(
        np.float32, copy=False
    )


def kernel(x1, x2) -> np.ndarray:
    from concourse.bass_utils import run_bass_kernel_spmd

    nc = get_nc()
    in_maps = make_in_maps(x1, x2)
    res = run_bass_kernel_spmd(nc, in_maps, list(range(NCORES)))
    return gather_out(res.results)


# revision 19
# speedup vs baseline: 1.0794x; 1.0193x over previous
"""Euclidean distance block (retrieval kNN) on 8 TRN2 NeuronCores.

dist[b, s, p] = sqrt(sum_c (x1[b, c, p] - x2[b, s, c, p])^2)   p = spatial (h*w)
out[b] = dist[b].reshape(S * h * w)

Sharding: data-parallel over batch B=32 -> 4 batches per core, no comms.

Per-core layout (spatial-split): SBUF partitions carry (channel, spatial_half)
= 64*2 = 128; the free axis carries the 882 spatial positions of one half.
Every tensor reshapes cleanly onto 128 partitions:
  x1[b]  -> [128, 882]      one DMA per batch, no partition-duplicate copy
  x2[b,s]-> [128, 882]      streamed as [128, 2, 882] two-support tiles
  out[b] -> rows (s, si)    [50, 882] f32, contiguous 3528 B per row

Compute chain per support: DVE subtract in bf16 (2x mode, in place), Square
split between ACT and DVE (cadence balance), then one [128, 50] one-hot mask
matmul per spatial half accumulating sum-over-C into PSUM [50, 441] (rows =
(s, si)); supports accumulate start/stop over s = 0..24.  LDWEIGHTS streams on
PE's second read port so per-support mask reloads pipeline behind the matmuls.
ACT Sqrt PSUM -> SBUF f32, one 176 KB store per batch on the otherwise-idle
Sync HWDGE ring (loads-with-cast must use the GpSimd SWDGE ring, and a store
queued behind loads on the same ring would stall them).
"""

import numpy as np

B, S, C, H, W = 32, 25, 64, 42, 42
HW = H * W            # 1764
PHALF = HW // 2       # 882
QW = PHALF // 2       # 441
NCORES = 8
BL = B // NCORES      # 4 batches per core
NSO = 2               # supports per streamed tile
NTILE = S // NSO      # 12 two-support tiles, then 1 leftover single

_cache = {}


def _build_nc():
    import concourse.bacc as bacc
    import concourse.mybir as mybir
    from concourse.tile import TileContext
    from concourse.bass import MemorySpace

    f32 = mybir.dt.float32
    bf16 = mybir.dt.bfloat16
    Square = mybir.ActivationFunctionType.Square
    Sqrt = mybir.ActivationFunctionType.Sqrt
    sub = mybir.AluOpType.subtract
    mult = mybir.AluOpType.mult

    # Square and Sqrt both live in the "sqrt_and_others" act-function set,
    # but the table-load chooser picks the first set containing each one,
    # alternating two ~2.7us table reloads per batch. Strip the two
    # functions from every other set (contents only — set ids are
    # positional) so one resident table serves the whole kernel.
    _orig_tables = bacc.get_activation_tables

    def _pinned_tables(arch):
        t = _orig_tables(arch)
        for name, fns in t.items():
            if name != "sqrt_and_others":
                fns.discard(Square)
                fns.discard(Sqrt)
        return t

    bacc.get_activation_tables = _pinned_tables
    nc = bacc.Bacc()
    # x1/x2 are staged to DRAM as bf16 by make_in_maps: the kernel is
    # HBM-stream-bound, so halving the bytes read halves the floor; the
    # compute pipeline already ran on bf16 (identical numerics to the
    # previous cast-during-DMA scheme).
    x1 = nc.declare_dram_parameter("x1", [BL, C, HW], bf16, isOutput=False)
    x2 = nc.declare_dram_parameter("x2", [BL, S, C, HW], bf16, isOutput=False)
    mk = nc.declare_dram_parameter("mask", [128, S, 2 * S], bf16, isOutput=False)
    out = nc.declare_dram_parameter("out", [BL, S * HW], f32, isOutput=True)

    with TileContext(nc) as tc:
        with (
            tc.tile_pool(name="x2p", bufs=16) as x2p,
            tc.tile_pool(name="x2lp", bufs=2) as x2lp,
            tc.tile_pool(name="sqp", bufs=8) as sqp,
            tc.tile_pool(name="x1p", bufs=2) as x1p,
            tc.tile_pool(name="outp", bufs=2) as outp,
            tc.tile_pool(name="cst", bufs=1) as cst,
            tc.tile_pool(name="ps", bufs=3, space=MemorySpace.PSUM) as psp,
        ):
            # mask rows are (s, si): mt[k, s, 2s+si(k)] = 1; contiguous
            # 2500 B per partition, one clean HWDGE load
            mt = cst.tile([128, S, 2 * S], bf16)
            nc.sync.dma_start(mt[:], mk[:, :, :])

            prev = None
            for b in range(BL):
                # x1[b]: partition (c, si), one contiguous 3528 B run per
                # partition, cast-loaded just ahead of its batch's stream
                x1bt = x1p.tile([128, PHALF], bf16, tag="x1b")
                nc.gpsimd.dma_start(
                    x1bt[:], x1[b].rearrange("c (si p) -> (c si) p", si=2)
                )
                # stream all supports of the batch: 12 two-support tiles
                # plus the odd support 24 as a single-support tile
                tiles = []
                for i in range(NTILE):
                    x2t = x2p.tile([128, NSO, PHALF], bf16, tag="x2t")
                    nc.gpsimd.dma_start(
                        x2t[:],
                        x2[b, NSO * i : NSO * (i + 1)].rearrange(
                            "s c (si p) -> (c si) s p", si=2
                        ),
                    )
                    tiles.append(x2t)
                x2l = x2lp.tile([128, 1, PHALF], bf16, tag="x2l")
                nc.gpsimd.dma_start(
                    x2l[:], x2[b, S - 1 :].rearrange("s c (si p) -> (c si) s p", si=2)
                )

                x1b = x1bt[:, :]
                pst = [
                    psp.tile([2 * S, QW], f32, name=f"ps{h}", tag=f"ps{h}")
                    for h in range(2)
                ]

                def flush_prev(prev=prev):
                    # emit the previous batch's sqrt + store mid-way into
                    # this batch's loop: by then its last matmul has long
                    # retired, so ACT slots the sqrts between squares
                    # instead of head-of-line blocking on the accumulation
                    if prev is None:
                        return
                    ppst, pb = prev
                    ot = outp.tile([2 * S, PHALF], f32, name="ot", tag="ot")
                    for h in range(2):
                        nc.scalar.activation(
                            ot[:, h * QW : (h + 1) * QW], ppst[h][:], Sqrt
                        )
                    nc.sync.dma_start(
                        out[pb].rearrange("(s si p) -> (s si) p", si=2, p=PHALF),
                        ot[:],
                    )

                x1bc = x1b.unsqueeze(1).broadcast_to([128, NSO, PHALF])
                for s in range(S):
                    if s == 4:
                        flush_prev()
                    if s < S - 1:
                        d = tiles[s // NSO][:, s % NSO, :]
                        if s % NSO == 0:
                            # one fused subtract per tile: x1 broadcast
                            # (stride-0) over the support dim halves the
                            # DVE op count
                            t = tiles[s // NSO]
                            nc.vector.tensor_tensor(t[:], t[:], x1bc, sub)
                    else:
                        d = x2l[:, 0, :]
                        nc.vector.tensor_tensor(d, d, x1b, sub)
                    sq = sqp.tile([128, PHALF], bf16, tag="sq")
                    # squares split 2/1 ACT/DVE to balance engine cadence
                    if s % 3 != 2:
                        nc.scalar.activation(sq[:], d, Square)
                    else:
                        nc.vector.tensor_tensor(sq[:], d, d, mult)
                    for h in range(2):
                        nc.tensor.matmul(
                            pst[h][:, :],
                            mt[:, s, :],
                            sq[:, h * QW : (h + 1) * QW],
                            start=(s == 0),
                            stop=(s == S - 1),
                        )
                prev = (pst, b)

            # final batch: sqrt + store immediately, split per half so the
            # h0 store overlaps the h1 sqrt
            ppst, pb = prev
            ot = outp.tile([2 * S, PHALF], f32, name="ot", tag="ot")
            dst = out[pb].rearrange("(s si p) -> (s si) p", si=2, p=PHALF)
            for h in range(2):
                nc.scalar.activation(ot[:, h * QW : (h + 1) * QW], ppst[h][:], Sqrt)
                nc.sync.dma_start(
                    dst[:, h * QW : (h + 1) * QW], ot[:, h * QW : (h + 1) * QW]
                )

    try:
        nc.finalize()
    finally:
        bacc.get_activation_tables = _orig_tables
    return nc


def get_nc():
    if "nc" not in _cache:
        _cache["nc"] = _build_nc()
    return _cache["nc"]


def make_mask() -> np.ndarray:
    # mask[k, s, m] = 1 iff the sum of partition k (= channel c = k//2,
    # spatial half si = k%2) for support s belongs to output row m = 2s + si.
    import ml_dtypes

    mask = np.zeros((128, S, 2 * S), dtype=ml_dtypes.bfloat16)
    for k in range(128):
        si = k % 2
        for s in range(S):
            mask[k, s, 2 * s + si] = 1.0
    return mask


def make_in_maps(x1: np.ndarray, x2: np.ndarray) -> list[dict]:
    import ml_dtypes

    bf = ml_dtypes.bfloat16
    x1 = np.ascontiguousarray(np.asarray(x1).astype(bf)).reshape(B, C, HW)
    x2 = np.ascontiguousarray(np.asarray(x2).astype(bf)).reshape(B, S, C, HW)
    mask = make_mask()
    maps = []
    for i in range(NCORES):
        sl = slice(i * BL, (i + 1) * BL)
        maps.append({"x1": x1[sl], "x2": x2[sl], "mask": mask})
    return maps


def gather_out(results: list[dict]) -> np.ndarray:
    return np.concatenate([np.asarray(r["out"]) for r in results], axis=0).astype(
        np.float32, copy=False
    )


def kernel(x1, x2) -> np.ndarray:
    from concourse.bass_utils import run_bass_kernel_spmd

    nc = get_nc()
    in_maps = make_in_maps(x1, x2)
    res = run_bass_kernel_spmd(nc, in_maps, list(range(NCORES)))
    return gather_out(res.results)


# revision 20
# speedup vs baseline: 1.1053x; 1.0240x over previous
"""Euclidean distance block (retrieval kNN) on 8 TRN2 NeuronCores.

dist[b, s, p] = sqrt(sum_c (x1[b, c, p] - x2[b, s, c, p])^2)   p = spatial (h*w)
out[b] = dist[b].reshape(S * h * w)

Sharding: data-parallel over batch B=32 -> 4 batches per core, no comms.

Per-core layout (spatial-split): SBUF partitions carry (channel, spatial_half)
= 64*2 = 128; the free axis carries the 882 spatial positions of one half.
Every tensor reshapes cleanly onto 128 partitions:
  x1[b]  -> [128, 882]      one DMA per batch, no partition-duplicate copy
  x2[b,s]-> [128, 882]      streamed as [128, 2, 882] two-support tiles
  out[b] -> rows (s, si)    [50, 882] f32, contiguous 3528 B per row

Compute chain per support: DVE subtract in bf16 (2x mode, in place), Square
split between ACT and DVE (cadence balance), then one [128, 50] one-hot mask
matmul per spatial half accumulating sum-over-C into PSUM [50, 441] (rows =
(s, si)); supports accumulate start/stop over s = 0..24.  LDWEIGHTS streams on
PE's second read port so per-support mask reloads pipeline behind the matmuls.
ACT Sqrt PSUM -> SBUF f32, one 176 KB store per batch on the otherwise-idle
Sync HWDGE ring (loads-with-cast must use the GpSimd SWDGE ring, and a store
queued behind loads on the same ring would stall them).
"""

import numpy as np

B, S, C, H, W = 32, 25, 64, 42, 42
HW = H * W            # 1764
PHALF = HW // 2       # 882
QW = PHALF // 2       # 441
NCORES = 8
BL = B // NCORES      # 4 batches per core
NSO = 2               # supports per streamed tile
NTILE = S // NSO      # 12 two-support tiles, then 1 leftover single

_cache = {}


def _build_nc():
    import concourse.bacc as bacc
    import concourse.mybir as mybir
    from concourse.tile import TileContext
    from concourse.bass import MemorySpace

    f32 = mybir.dt.float32
    bf16 = mybir.dt.bfloat16
    Square = mybir.ActivationFunctionType.Square
    Sqrt = mybir.ActivationFunctionType.Sqrt
    sub = mybir.AluOpType.subtract
    mult = mybir.AluOpType.mult

    # Square and Sqrt both live in the "sqrt_and_others" act-function set,
    # but the table-load chooser picks the first set containing each one,
    # alternating two ~2.7us table reloads per batch. Strip the two
    # functions from every other set (contents only — set ids are
    # positional) so one resident table serves the whole kernel.
    _orig_tables = bacc.get_activation_tables

    def _pinned_tables(arch):
        t = _orig_tables(arch)
        for name, fns in t.items():
            if name != "sqrt_and_others":
                fns.discard(Square)
                fns.discard(Sqrt)
        return t

    bacc.get_activation_tables = _pinned_tables
    nc = bacc.Bacc()
    # x1/x2 are staged to DRAM as bf16 by make_in_maps: the kernel is
    # HBM-stream-bound, so halving the bytes read halves the floor; the
    # compute pipeline already ran on bf16 (identical numerics to the
    # previous cast-during-DMA scheme).
    x1 = nc.declare_dram_parameter("x1", [BL, C, HW], bf16, isOutput=False)
    x2 = nc.declare_dram_parameter("x2", [BL, S, C, HW], bf16, isOutput=False)
    mk = nc.declare_dram_parameter("mask", [128, S, 2 * S], bf16, isOutput=False)
    out = nc.declare_dram_parameter("out", [BL, S * HW], f32, isOutput=True)

    with TileContext(nc) as tc:
        with (
            tc.tile_pool(name="x2p", bufs=16) as x2p,
            tc.tile_pool(name="x2lp", bufs=2) as x2lp,
            tc.tile_pool(name="sqp", bufs=8) as sqp,
            tc.tile_pool(name="x1p", bufs=2) as x1p,
            tc.tile_pool(name="outp", bufs=2) as outp,
            tc.tile_pool(name="cst", bufs=1) as cst,
            tc.tile_pool(name="ps", bufs=3, space=MemorySpace.PSUM) as psp,
        ):
            # mask rows are (s, si): mt[k, s, 2s+si(k)] = 1; contiguous
            # 2500 B per partition, one clean HWDGE load
            mt = cst.tile([128, S, 2 * S], bf16)
            nc.sync.dma_start(mt[:], mk[:, :, :])

            prev = None
            for b in range(BL):
                # x1[b]: partition (c, si), one contiguous 3528 B run per
                # partition, cast-loaded just ahead of its batch's stream
                x1bt = x1p.tile([128, PHALF], bf16, tag="x1b")
                nc.gpsimd.dma_start(
                    x1bt[:], x1[b].rearrange("c (si p) -> (c si) p", si=2)
                )
                # stream all supports of the batch: 12 two-support tiles
                # plus the odd support 24 as a single-support tile
                tiles = []
                for i in range(NTILE):
                    x2t = x2p.tile([128, NSO, PHALF], bf16, tag="x2t")
                    nc.gpsimd.dma_start(
                        x2t[:],
                        x2[b, NSO * i : NSO * (i + 1)].rearrange(
                            "s c (si p) -> (c si) s p", si=2
                        ),
                    )
                    tiles.append(x2t)
                x2l = x2lp.tile([128, 1, PHALF], bf16, tag="x2l")
                nc.gpsimd.dma_start(
                    x2l[:], x2[b, S - 1 :].rearrange("s c (si p) -> (c si) s p", si=2)
                )

                x1b = x1bt[:, :]
                pst = [
                    psp.tile([2 * S, QW], f32, name=f"ps{h}", tag=f"ps{h}")
                    for h in range(2)
                ]

                HQ = QW // 2
                fl = {"ot": None}

                def flush_chunk(c, prev=prev, fl=fl):
                    # spread the previous batch's sqrt over four quarter
                    # chunks, emitted at supports whose square went to DVE:
                    # ACT is free there, so the sqrts slot in without
                    # stalling the square pipeline (a single 1.2us sqrt
                    # lump at one spot cost ~2us of pipeline hiccup)
                    if prev is None:
                        return
                    ppst, pb = prev
                    if fl["ot"] is None:
                        fl["ot"] = outp.tile([2 * S, PHALF], f32, name="ot", tag="ot")
                    ot = fl["ot"]
                    h, q = c // 2, c % 2
                    w = HQ if q == 0 else QW - HQ
                    osl = slice(h * QW + q * HQ, h * QW + q * HQ + w)
                    psl = slice(q * HQ, q * HQ + w)
                    nc.scalar.activation(ot[:, osl], ppst[h][:, psl], Sqrt)
                    if c == 3:
                        nc.sync.dma_start(
                            out[pb].rearrange("(s si p) -> (s si) p", si=2, p=PHALF),
                            ot[:],
                        )

                x1bc = x1b.unsqueeze(1).broadcast_to([128, NSO, PHALF])
                for s in range(S):
                    if s in (5, 8, 11, 14):
                        flush_chunk((s - 5) // 3)
                    if s < S - 1:
                        d = tiles[s // NSO][:, s % NSO, :]
                        if s % NSO == 0:
                            # one fused subtract per tile: x1 broadcast
                            # (stride-0) over the support dim halves the
                            # DVE op count
                            t = tiles[s // NSO]
                            nc.vector.tensor_tensor(t[:], t[:], x1bc, sub)
                    else:
                        d = x2l[:, 0, :]
                        nc.vector.tensor_tensor(d, d, x1b, sub)
                    sq = sqp.tile([128, PHALF], bf16, tag="sq")
                    # squares split 2/1 ACT/DVE to balance engine cadence
                    if s % 3 != 2:
                        nc.scalar.activation(sq[:], d, Square)
                    else:
                        nc.vector.tensor_tensor(sq[:], d, d, mult)
                    for h in range(2):
                        nc.tensor.matmul(
                            pst[h][:, :],
                            mt[:, s, :],
                            sq[:, h * QW : (h + 1) * QW],
                            start=(s == 0),
                            stop=(s == S - 1),
                        )
                prev = (pst, b)

            # final batch: sqrt + store immediately, split per half so the
            # h0 store overlaps the h1 sqrt
            ppst, pb = prev
            ot = outp.tile([2 * S, PHALF], f32, name="ot", tag="ot")
            dst = out[pb].rearrange("(s si p) -> (s si) p", si=2, p=PHALF)
            for h in range(2):
                nc.scalar.activation(ot[:, h * QW : (h + 1) * QW], ppst[h][:], Sqrt)
                nc.sync.dma_start(
                    dst[:, h * QW : (h + 1) * QW], ot[:, h * QW : (h + 1) * QW]
                )

    try:
        nc.finalize()
    finally:
        bacc.get_activation_tables = _orig_tables
    return nc


def get_nc():
    if "nc" not in _cache:
        _cache["nc"] = _build_nc()
    return _cache["nc"]


def make_mask() -> np.ndarray:
    # mask[k, s, m] = 1 iff the sum of partition k (= channel c = k//2,
    # spatial half si = k%2) for support s belongs to output row m = 2s + si.
    import ml_dtypes

    mask = np.zeros((128, S, 2 * S), dtype=ml_dtypes.bfloat16)
    for k in range(128):
        si = k % 2
        for s in range(S):
            mask[k, s, 2 * s + si] = 1.0
    return mask


def make_in_maps(x1: np.ndarray, x2: np.ndarray) -> list[dict]:
    import ml_dtypes

    bf = ml_dtypes.bfloat16
    x1 = np.ascontiguousarray(np.asarray(x1).astype(bf)).reshape(B, C, HW)
    x2 = np.ascontiguousarray(np.asarray(x2).astype(bf)).reshape(B, S, C, HW)
    mask = make_mask()
    maps = []
    for i in range(NCORES):
        sl = slice(i * BL, (i + 1) * BL)
        maps.append({"x1": x1[sl], "x2": x2[sl], "mask": mask})
    return maps


def gather_out(results: list[dict]) -> np.ndarray:
    return np.concatenate([np.asarray(r["out"]) for r in results], axis=0).astype(
        np.float32, copy=False
    )


def kernel(x1, x2) -> np.ndarray:
    from concourse.bass_utils import run_bass_kernel_spmd

    nc = get_nc()
    in_maps = make_in_maps(x1, x2)
    res = run_bass_kernel_spmd(nc, in_maps, list(range(NCORES)))
    return gather_out(res.results)


# revision 21
# speedup vs baseline: 1.1083x; 1.0027x over previous
"""Euclidean distance block (retrieval kNN) on 8 TRN2 NeuronCores.

dist[b, s, p] = sqrt(sum_c (x1[b, c, p] - x2[b, s, c, p])^2)   p = spatial (h*w)
out[b] = dist[b].reshape(S * h * w)

Sharding: data-parallel over batch B=32 -> 4 batches per core, no comms.

Per-core layout (spatial-split): SBUF partitions carry (channel, spatial_half)
= 64*2 = 128; the free axis carries the 882 spatial positions of one half.
Every tensor reshapes cleanly onto 128 partitions:
  x1[b]  -> [128, 882]      one DMA per batch, no partition-duplicate copy
  x2[b,s]-> [128, 882]      streamed as [128, 2, 882] two-support tiles
  out[b] -> rows (s, si)    [50, 882] f32, contiguous 3528 B per row

Inputs are restaged to DRAM as bf16 by make_in_maps (the kernel is
HBM-stream-bound; halving the bytes read halves the floor, and the compute
pipeline is bf16 anyway — numerically identical to a cast-during-DMA load).

Compute chain: one fused DVE subtract per 2-support tile (bf16 2x mode, in
place, x1 stride-0-broadcast over the support dim), Square split 2/1
ACT/DVE (cadence balance), then one [128, 50] one-hot mask matmul per
spatial half accumulating sum-over-C into PSUM [50, 441] (rows = (s, si));
supports accumulate start/stop over s = 0..24. LDWEIGHTS streams on PE's
second read port so per-support mask reloads pipeline behind the matmuls.
The previous batch's ACT Sqrt is spread over four quarter-chunks emitted at
supports whose square went to DVE (ACT is free there — a single sqrt lump
cost ~2us of pipeline hiccup per batch), then one 176 KB store per batch on
the otherwise-idle Sync HWDGE ring (a store queued behind the loads on the
gpsimd ring would head-of-line block them).

Known pacer: SDMA engine 15 runs ~6% slower than its peers (both DGE
paths), and every tile's completion sem waits on all 16 engines, so the
stream tail inherits its straggle; the deep x2 pool (16 tiles) absorbs the
jitter mid-stream.
"""

import numpy as np

B, S, C, H, W = 32, 25, 64, 42, 42
HW = H * W            # 1764
PHALF = HW // 2       # 882
QW = PHALF // 2       # 441
NCORES = 8
BL = B // NCORES      # 4 batches per core
NSO = 2               # supports per streamed tile
NTILE = S // NSO      # 12 two-support tiles, then 1 leftover single

_cache = {}


def _build_nc():
    import concourse.bacc as bacc
    import concourse.mybir as mybir
    from concourse.tile import TileContext
    from concourse.bass import MemorySpace

    f32 = mybir.dt.float32
    bf16 = mybir.dt.bfloat16
    Square = mybir.ActivationFunctionType.Square
    Sqrt = mybir.ActivationFunctionType.Sqrt
    sub = mybir.AluOpType.subtract
    mult = mybir.AluOpType.mult

    # Square and Sqrt both live in the "sqrt_and_others" act-function set,
    # but the table-load chooser picks the first set containing each one,
    # alternating two ~2.7us table reloads per batch. Strip the two
    # functions from every other set (contents only — set ids are
    # positional) so one resident table serves the whole kernel.
    _orig_tables = bacc.get_activation_tables

    def _pinned_tables(arch):
        t = _orig_tables(arch)
        for name, fns in t.items():
            if name != "sqrt_and_others":
                fns.discard(Square)
                fns.discard(Sqrt)
        return t

    bacc.get_activation_tables = _pinned_tables
    nc = bacc.Bacc()
    # x1/x2 are staged to DRAM as bf16 by make_in_maps: the kernel is
    # HBM-stream-bound, so halving the bytes read halves the floor; the
    # compute pipeline already ran on bf16 (identical numerics to the
    # previous cast-during-DMA scheme).
    x1 = nc.declare_dram_parameter("x1", [BL, C, HW], bf16, isOutput=False)
    x2 = nc.declare_dram_parameter("x2", [BL, S, C, HW], bf16, isOutput=False)
    mk = nc.declare_dram_parameter("mask", [128, S, 2 * S], bf16, isOutput=False)
    out = nc.declare_dram_parameter("out", [BL, S * HW], f32, isOutput=True)

    with TileContext(nc) as tc:
        with (
            tc.tile_pool(name="x2p", bufs=16) as x2p,
            tc.tile_pool(name="x2lp", bufs=2) as x2lp,
            tc.tile_pool(name="sqp", bufs=8) as sqp,
            tc.tile_pool(name="x1p", bufs=2) as x1p,
            tc.tile_pool(name="outp", bufs=2) as outp,
            tc.tile_pool(name="cst", bufs=1) as cst,
            tc.tile_pool(name="ps", bufs=3, space=MemorySpace.PSUM) as psp,
        ):
            # mask rows are (s, si): mt[k, s, 2s+si(k)] = 1; contiguous
            # 2500 B per partition, one clean HWDGE load
            mt = cst.tile([128, S, 2 * S], bf16)
            nc.sync.dma_start(mt[:], mk[:, :, :])

            prev = None
            for b in range(BL):
                # x1[b]: partition (c, si), one contiguous 3528 B run per
                # partition, cast-loaded just ahead of its batch's stream
                x1bt = x1p.tile([128, PHALF], bf16, tag="x1b")
                nc.gpsimd.dma_start(
                    x1bt[:], x1[b].rearrange("c (si p) -> (c si) p", si=2)
                )
                # stream all supports of the batch: 12 two-support tiles
                # plus the odd support 24 as a single-support tile
                tiles = []
                for i in range(NTILE):
                    x2t = x2p.tile([128, NSO, PHALF], bf16, tag="x2t")
                    nc.gpsimd.dma_start(
                        x2t[:],
                        x2[b, NSO * i : NSO * (i + 1)].rearrange(
                            "s c (si p) -> (c si) s p", si=2
                        ),
                    )
                    tiles.append(x2t)
                x2l = x2lp.tile([128, 1, PHALF], bf16, tag="x2l")
                nc.gpsimd.dma_start(
                    x2l[:], x2[b, S - 1 :].rearrange("s c (si p) -> (c si) s p", si=2)
                )

                x1b = x1bt[:, :]
                pst = [
                    psp.tile([2 * S, QW], f32, name=f"ps{h}", tag=f"ps{h}")
                    for h in range(2)
                ]

                HQ = QW // 2
                fl = {"ot": None}

                def flush_chunk(c, prev=prev, fl=fl):
                    # spread the previous batch's sqrt over four quarter
                    # chunks, emitted at supports whose square went to DVE:
                    # ACT is free there, so the sqrts slot in without
                    # stalling the square pipeline (a single 1.2us sqrt
                    # lump at one spot cost ~2us of pipeline hiccup)
                    if prev is None:
                        return
                    ppst, pb = prev
                    if fl["ot"] is None:
                        fl["ot"] = outp.tile([2 * S, PHALF], f32, name="ot", tag="ot")
                    ot = fl["ot"]
                    h, q = c // 2, c % 2
                    w = HQ if q == 0 else QW - HQ
                    osl = slice(h * QW + q * HQ, h * QW + q * HQ + w)
                    psl = slice(q * HQ, q * HQ + w)
                    nc.scalar.activation(ot[:, osl], ppst[h][:, psl], Sqrt)
                    if c == 3:
                        nc.sync.dma_start(
                            out[pb].rearrange("(s si p) -> (s si) p", si=2, p=PHALF),
                            ot[:],
                        )

                x1bc = x1b.unsqueeze(1).broadcast_to([128, NSO, PHALF])
                for s in range(S):
                    if s in (5, 8, 11, 14):
                        flush_chunk((s - 5) // 3)
                    if s < S - 1:
                        d = tiles[s // NSO][:, s % NSO, :]
                        if s % NSO == 0:
                            # one fused subtract per tile: x1 broadcast
                            # (stride-0) over the support dim halves the
                            # DVE op count
                            t = tiles[s // NSO]
                            nc.vector.tensor_tensor(t[:], t[:], x1bc, sub)
                    else:
                        d = x2l[:, 0, :]
                        nc.vector.tensor_tensor(d, d, x1b, sub)
                    sq = sqp.tile([128, PHALF], bf16, tag="sq")
                    # squares split 2/1 ACT/DVE to balance engine cadence
                    if s % 3 != 2:
                        nc.scalar.activation(sq[:], d, Square)
                    else:
                        nc.vector.tensor_tensor(sq[:], d, d, mult)
                    for h in range(2):
                        nc.tensor.matmul(
                            pst[h][:, :],
                            mt[:, s, :],
                            sq[:, h * QW : (h + 1) * QW],
                            start=(s == 0),
                            stop=(s == S - 1),
                        )
                prev = (pst, b)

            # final batch: sqrt + store immediately, split per half so the
            # h0 store overlaps the h1 sqrt
            ppst, pb = prev
            ot = outp.tile([2 * S, PHALF], f32, name="ot", tag="ot")
            dst = out[pb].rearrange("(s si p) -> (s si) p", si=2, p=PHALF)
            for h in range(2):
                nc.scalar.activation(ot[:, h * QW : (h + 1) * QW], ppst[h][:], Sqrt)
                nc.sync.dma_start(
                    dst[:, h * QW : (h + 1) * QW], ot[:, h * QW : (h + 1) * QW]
                )

    try:
        nc.finalize()
    finally:
        bacc.get_activation_tables = _orig_tables
    return nc


def get_nc():
    if "nc" not in _cache:
        _cache["nc"] = _build_nc()
    return _cache["nc"]


def make_mask() -> np.ndarray:
    # mask[k, s, m] = 1 iff the sum of partition k (= channel c = k//2,
    # spatial half si = k%2) for support s belongs to output row m = 2s + si.
    import ml_dtypes

    mask = np.zeros((128, S, 2 * S), dtype=ml_dtypes.bfloat16)
    for k in range(128):
        si = k % 2
        for s in range(S):
            mask[k, s, 2 * s + si] = 1.0
    return mask


def make_in_maps(x1: np.ndarray, x2: np.ndarray) -> list[dict]:
    import ml_dtypes

    bf = ml_dtypes.bfloat16
    x1 = np.ascontiguousarray(np.asarray(x1).astype(bf)).reshape(B, C, HW)
    x2 = np.ascontiguousarray(np.asarray(x2).astype(bf)).reshape(B, S, C, HW)
    mask = make_mask()
    maps = []
    for i in range(NCORES):
        sl = slice(i * BL, (i + 1) * BL)
        maps.append({"x1": x1[sl], "x2": x2[sl], "mask": mask})
    return maps


def gather_out(results: list[dict]) -> np.ndarray:
    return np.concatenate([np.asarray(r["out"]) for r in results], axis=0).astype(
        np.float32, copy=False
    )


def kernel(x1, x2) -> np.ndarray:
    from concourse.bass_utils import run_bass_kernel_spmd

    nc = get_nc()
    in_maps = make_in_maps(x1, x2)
    res = run_bass_kernel_spmd(nc, in_maps, list(range(NCORES)))
    return gather_out(res.results)
